# revision 1
# baseline (speedup 1.0000x reference)
"""Trainium2 Bass kernel for nn_Attention_32049045963483 (sparse_attention).

Math collapse (verified vs reference at ~3e-6 rel err):
  - qkv 1x1 conv folds into the 11x11/stride-8 down-convs:
      conv(W1 @ f, wq) == conv(f, w_eff),  w_eff[oc,d] = sum_ic wq[oc,ic] W1[ic,d]
  - nearest-neighbor 64x upsample of the [64,64] score map + softmax over the
    upsampled axis == softmax of the low-res map; with row index i -> i//64 = x,
    every output row depends only on x.
  - v enters only through 64-wide block sums:  vbar[c,J] = sum_y v[c,J,y]
      = Wv @ fbar,  fbar[d,J] = sum_y f[d,J,y]   (v never materializes)
  - out[c,x,y] = (sum_J e[J,x] * vbar[c,J]) / (64 * sum_J e[J,x]),
      e[J,I] = exp(scale * dots[I,J])  -- broadcast along y.

Sharding: head-parallel over 8 cores. Core i computes global channels
8i..8i+7 (head i): conv out-channel slices of wq/wk, v-row slice of w_qkv.
Each core reads full f (the down-convs mix all 64 input channels).

Conv structure: factorized two-stage form so the heavy matmuls stream with
free dim >= 256, where float32r runs at full rate (plain fp32 is 1/4):
  stage 1: s[(ky,oc), r, ox] = sum_d w_eff[d,(ky,oc)]@kx . fpad[d, r, 8ox+kx]
           accumulated over kx, in two r-chunks (B: rows 34..66 first --
           matches DMA arrival order -- then A: rows 0..33)
  stage 2: q_low[oc,(oy,ox)]  = sum_ky s[(ky,oc), 8oy+ky, ox]
           via identity-slice selection stationaries; q and k accumulate in
           separate base-0 PSUM tiles so dots needs no partition rebase.
"""

import numpy as np

N_CORES = 8
SCALE = 8.0 ** -0.5  # dim_head ** -0.5

# packed [64, *] weight tensor columns: [wqr | w1q | wkr | w1k | wvt | bq | bk]
C_WQR = 0
C_W1Q = 968
C_WKR = 1032
C_W1K = 2000
C_WVT = 2064
C_BQ = 2072
C_BK = 2073
C_TOT = 2080

_CACHE = {}

LAST_RESULTS = None  # BassKernelResults of the most recent run (for test harness)


def _dep(after, before, sync=False):
    from concourse.tile import add_dep_helper

    a = getattr(after, "ins", after)
    b = getattr(before, "ins", before)
    add_dep_helper(a, b, sync=sync, reason="pin order")


def _build_nc():
    from contextlib import ExitStack

    import concourse.bacc as bacc
    import concourse.mybir as mybir
    import concourse.tile as tile

    f32 = mybir.dt.float32
    f32r = mybir.dt.float32r
    bf16 = mybir.dt.bfloat16
    X = mybir.AxisListType.X
    AF = mybir.ActivationFunctionType

    # Bacc (not raw Bass): its compile() splits >1-wait sync via event
    # semaphores -- hardware allows only one sync wait per instruction.
    nc = bacc.Bacc("TRN2", target_bir_lowering=False)

    f_d = nc.dram_tensor("f", [64, 68 * 68], f32r, kind="ExternalInput")
    wp_d = nc.dram_tensor("wp", [64, 2064], mybir.dt.float16, kind="ExternalInput")
    w2_d = nc.dram_tensor("w2", [64, 16], f32, kind="ExternalInput")
    ws_d = nc.dram_tensor("ws", [88, 88], f32r, kind="ExternalInput")
    out_d = nc.dram_tensor("out", [8, 4096], f32, kind="ExternalOutput")

    with tile.TileContext(nc) as tc:
        with ExitStack() as ctx:
            sb = ctx.enter_context(tc.tile_pool(name="sb", bufs=1))
            ps = ctx.enter_context(tc.tile_pool(name="ps", bufs=1, space="PSUM"))

            fpad = sb.tile([64, 68 * 68], f32r)
            wp_t = sb.tile([64, 2064], mybir.dt.float16)
            w2_t = sb.tile([64, 16], f32)
            ws_t = sb.tile([88, 88], f32r)
            wmq_t = sb.tile([64, 968], f32r)
            wmk_t = sb.tile([64, 968], f32r)
            sq_t = sb.tile([88, 536], f32r)
            sk_t = sb.tile([88, 536], f32r)
            q_t = sb.tile([8, 64], f32)
            k_t = sb.tile([8, 64], f32)
            e_t = sb.tile([64, 64], f32)
            fbar_t = sb.tile([64, 64], f32)
            vaug_t = sb.tile([64, 9], f32)
            rs_t = sb.tile([64, 1], f32)
            olT_t = sb.tile([64, 8], f32)
            T_t = sb.tile([64, 8 * 64], f32)
            scr_t = sb.tile([1, 1], f32)
            scr2_t = sb.tile([1, 1], f32)

            # --- input DMAs interleaved across both HWDGE rings in priority
            # order: conv weights, f rows 0..33 (A-chunks), f rows 34..67, ws
            fp3 = fpad.rearrange("p (r c) -> p r c", c=68)
            # Strict arrival priority at full 2-ring bandwidth: each f piece's
            # trigger waits on the previous priority class's completion on the
            # OTHER ring (same-ring order is implicit), so wp lands first,
            # then f rows 0..33 (A-chunks), then rows 34..67.
            d_wpA = nc.sync.dma_start(out=wp_t[:, 0:C_WKR], in_=wp_d[:, 0:C_WKR])
            d_wpB = nc.scalar.dma_start(out=wp_t[:, C_WKR:2064], in_=wp_d[:, C_WKR:2064])
            d_fA1 = nc.sync.dma_start(out=fpad[:, 0:1156], in_=f_d[:, 0:1156])
            d_fA2 = nc.scalar.dma_start(out=fpad[:, 1156:2312], in_=f_d[:, 1156:2312])
            d_fB1 = nc.sync.dma_start(out=fpad[:, 2312:3468], in_=f_d[:, 2312:3468])
            d_fB2 = nc.scalar.dma_start(out=fpad[:, 3468:4624], in_=f_d[:, 3468:4624])
            d_ws = nc.sync.dma_start(out=ws_t, in_=ws_d[:])
            d_w2 = nc.sync.dma_start(out=w2_t, in_=w2_d[:])

            # preload ACT function tables during the DMA wait (after the ACT
            # ring's DMA triggers). Exp first, Gelu LAST so the gelu set is
            # resident for the real GELUs; the exp reload hides behind dots.
            nc.vector.memset(scr_t, 0.0)
            nc.vector.memset(vaug_t[:, 8:9], 64.0)
            de = nc.scalar.activation(out=scr2_t, in_=scr_t, func=AF.Exp)
            dg = nc.scalar.activation(out=scr2_t, in_=scr_t, func=AF.Gelu)
            _dep(de, d_wpB)
            _dep(de, d_fB2)
            _dep(dg, de)

            wqr4 = wp_t[:, C_WQR:C_W1Q].rearrange(
                "p (kx ky oc) -> p kx ky oc", ky=11, oc=8
            )
            wkr4 = wp_t[:, C_WKR:C_W1K].rearrange(
                "p (kx ky oc) -> p kx ky oc", ky=11, oc=8
            )
            w1q = wp_t[:, C_W1Q:C_WKR]
            w1k = wp_t[:, C_W1K:C_WVT]
            wvt_v = w2_t[:, 0:8]
            bq_v = w2_t[0:8, 8:9]
            bk_v = w2_t[0:8, 9:10]

            # --- compose conv weights: w_eff[d,(kx,ky,oc)], f32r big-free MMs
            psq = ps.tile([64, 11 * 128], f32, tag="A")
            psk = ps.tile([64, 11 * 128], f32, tag="B")
            psq4 = psq.rearrange("p (kx pad) -> p kx pad", pad=128)
            psk4 = psk.rearrange("p (kx pad) -> p kx pad", pad=128)

            def compose(ps4, w1, wr4):
                for x0, x1 in ((0, 4), (4, 8), (8, 11)):
                    nc.tensor.matmul(
                        ps4[:, x0:x1, 0:88], w1, wr4[:, x0:x1],
                        start=True, stop=True,
                    )

            compose(psq4, w1q, wqr4)
            nc.vector.tensor_copy(out=wmq_t, in_=psq4[:, :, 0:88])
            compose(psk4, w1k, wkr4)
            nc.vector.tensor_copy(out=wmk_t, in_=psk4[:, :, 0:88])

            # --- stage 1: per conv, 11 kx accumulate; free = (r-chunk, ox)
            def s1(pst, wm, sl_r):
                out = []
                for kx in range(11):
                    out.append(nc.tensor.matmul(
                        pst, wm[:, kx * 88 : kx * 88 + 88],
                        fp3[:, sl_r, kx : kx + 57 : 8],
                        start=(kx == 0), stop=(kx == 10),
                    ))
                return out[0]

            slAr, slBr = slice(0, 34), slice(34, 67)

            # fbar sub-reduces interleave into DVE gaps (J in groups of 16);
            # parts 0,1 need only f rows 2..33 which arrive first
            def fbar_part(j):
                return nc.vector.reduce_sum(
                    out=fbar_t[:, 16 * j : 16 * (j + 1)],
                    in_=fp3[:, 2 + 16 * j : 18 + 16 * j, 2:66].bitcast(f32),
                    axis=X,
                )

            gateA = nc.tensor.ldweights(weights=fpad[:, 138:139].bitcast(bf16))
            gateA2 = nc.tensor.ldweights(weights=fpad[:, 1160:1161].bitcast(bf16))
            ps_qA = ps.tile([88, 272], f32, tag="C")
            ps_kA = ps.tile([88, 272], f32, tag="D")
            qa = s1(ps_qA, wmq_t, slAr)
            _dep(qa, gateA)
            _dep(qa, gateA2)
            fb0 = fbar_part(0)
            fb1 = fbar_part(1)
            cast_qA = nc.vector.tensor_copy(out=sq_t[:, 0:272], in_=ps_qA)
            ka = s1(ps_kA, wmk_t, slAr)
            cast_kA = nc.vector.tensor_copy(out=sk_t[:, 0:272], in_=ps_kA)

            gateB = nc.tensor.ldweights(weights=fpad[:, 2316:2317].bitcast(bf16))
            gateB2 = nc.tensor.ldweights(weights=fpad[:, 3473:3474].bitcast(bf16))
            ps_qB = ps.tile([88, 264], f32, tag="A")
            ps_kB = ps.tile([88, 264], f32, tag="B")
            qb = s1(ps_qB, wmq_t, slBr)
            _dep(qb, gateB)
            _dep(qb, gateB2)
            fb2 = fbar_part(2)
            fb3 = fbar_part(3)
            cast_qB = nc.vector.tensor_copy(out=sq_t[:, 272:536], in_=ps_qB)
            kb = s1(ps_kB, wmk_t, slBr)
            cast_kB = nc.vector.tensor_copy(out=sk_t[:, 272:536], in_=ps_kB)

            # --- stage 2: k then q, separate base-0 PSUM accumulators
            sq3 = sq_t.rearrange("p (rr ox) -> p rr ox", ox=8)
            sk3 = sk_t.rearrange("p (rr ox) -> p rr ox", ox=8)
            # q group first: its cast lands earlier, so PE streams stage-2-q
            # while the k-conv's last PSUM cast is still finishing on DVE
            psc_k = ps.tile([8, 64], f32, tag="D")
            psc_q = ps.tile([8, 64], f32, tag="C")
            for ky in range(11):
                nc.tensor.matmul(
                    psc_q, ws_t[:, ky * 8 : ky * 8 + 8],
                    sq3[:, ky : ky + 57 : 8, :],
                    start=(ky == 0), stop=(ky == 10),
                )
            nc.scalar.activation(
                out=q_t, in_=psc_q, func=AF.Gelu, bias=bq_v, scale=1.0
            )
            for ky in range(11):
                nc.tensor.matmul(
                    psc_k, ws_t[:, ky * 8 : ky * 8 + 8],
                    sk3[:, ky : ky + 57 : 8, :],
                    start=(ky == 0), stop=(ky == 10),
                )
            nc.scalar.activation(
                out=k_t, in_=psc_k, func=AF.Gelu, bias=bk_v, scale=1.0
            )

            # --- vbar path (fbar parts already reduced during stage 1)
            gate_v = nc.tensor.ldweights(weights=fbar_t[:, 0:1].bitcast(bf16))
            gate_v2 = nc.tensor.ldweights(weights=fbar_t[:, 63:64].bitcast(bf16))
            psv = ps.tile([64, 8], f32, tag="A")
            vmm = nc.tensor.matmul(
                psv, fbar_t, wvt_v, start=True, stop=True
            )
            _dep(vmm, gate_v)
            _dep(vmm, gate_v2)
            nc.scalar.copy(out=vaug_t[:, 0:8], in_=psv)

            # --- dots_T[J,I] = sum_c k[c,J] q[c,I];  e = exp(scale * dots_T)
            gate2 = nc.tensor.ldweights(weights=k_t[:, 0:1].bitcast(bf16))
            psd = ps.tile([64, 64], f32, tag="B")
            dmm = nc.tensor.matmul(psd, k_t, q_t, start=True, stop=True)
            _dep(dmm, gate2)
            nc.scalar.activation(out=e_t, in_=psd, func=AF.Exp, scale=SCALE)

            # --- out_u[I, 0:8] = sum_J e[J,I] vbar[J,c]; col 8 = 64*sum_J e
            gate_o = nc.tensor.ldweights(weights=e_t[:, 0:1].bitcast(bf16))
            pso = ps.tile([64, 9], f32, tag="A")
            omm = nc.tensor.matmul(pso, e_t, vaug_t, start=True, stop=True)
            _dep(omm, gate_o)
            nc.vector.reciprocal(out=rs_t, in_=pso[:, 8:9])
            nc.vector.tensor_scalar_mul(olT_t, pso[:, 0:8], rs_t)

            # --- broadcast along y: single DVE copy with stride-0 read on y
            import concourse.bass as bass
            T3 = T_t.rearrange("p (c y) -> p c y", y=64)
            ola = olT_t[:]
            ol_b = bass.AP(
                tensor=ola.tensor, offset=ola.offset,
                ap=[list(ola.ap[0]), list(ola.ap[1]), [0, 64]],
            )
            nc.vector.tensor_copy(out=T3, in_=ol_b)

            # --- store: out[c, x, y] <- T[x, c, y]
            out_ap = out_d[:].rearrange("c (x y) -> c x y", y=64).transpose([1, 0, 2])
            nc.sync.dma_start(out=out_ap, in_=T3)

    nc.finalize()
    return nc


def _get_nc():
    if "nc" not in _CACHE:
        _CACHE["nc"] = _build_nc()
    return _CACHE["nc"]


_WSEL = np.eye(88, dtype=np.float32)


def kernel(**inputs):
    global LAST_RESULTS
    from concourse.bass_utils import run_bass_kernel_spmd

    f = np.ascontiguousarray(inputs["f"], np.float32)
    w_qkv = np.ascontiguousarray(inputs["w_qkv"], np.float32)[:, :, 0, 0]  # [192,64]
    wq = np.ascontiguousarray(inputs["wq"], np.float32)
    wk = np.ascontiguousarray(inputs["wk"], np.float32)
    bq = np.ascontiguousarray(inputs["bq"], np.float32)
    bk = np.ascontiguousarray(inputs["bk"], np.float32)

    f2 = np.zeros((64, 68, 68), np.float32)
    f2[:, 2:66, 2:66] = f[0]
    f2 = f2.reshape(64, 68 * 68)

    in_maps = []
    for i in range(N_CORES):
        sl = slice(8 * i, 8 * i + 8)
        wp = np.zeros((64, 2064), np.float16)
        # [oc,ic,ky,kx] slice -> [ic,kx,ky,oc]
        wp[:, C_WQR:C_W1Q] = wq[sl].transpose(1, 3, 2, 0).reshape(64, 968)
        wp[:, C_W1Q:C_WKR] = w_qkv[0:64]
        wp[:, C_WKR:C_W1K] = wk[sl].transpose(1, 3, 2, 0).reshape(64, 968)
        wp[:, C_W1K:C_WVT] = w_qkv[64:128]
        w2 = np.zeros((64, 16), np.float32)
        w2[:, 0:8] = w_qkv[128 + 8 * i : 136 + 8 * i].T
        w2[0:8, 8] = bq[sl]
        w2[0:8, 9] = bk[sl]
        in_maps.append({"f": f2, "wp": wp, "w2": w2, "ws": _WSEL})

    nc = _get_nc()
    res = run_bass_kernel_spmd(nc, in_maps, core_ids=list(range(N_CORES)))
    LAST_RESULTS = res
    out = np.concatenate([r["out"] for r in res.results], axis=0)  # [64, 4096]
    return out.reshape(1, 64, 64, 64)



# revision 8
# speedup vs baseline: 1.2180x; 1.2180x over previous
"""Trainium2 Bass kernel for nn_Attention_32049045963483 (sparse_attention).

Math collapse (verified vs reference: ~3e-6 rel err fp32, ~4.6e-4 fp16):
  - qkv 1x1 conv folds into the 11x11/stride-8 down-convs (host-side fold):
      w_eff[d, ky, kx, oc] = sum_ic w[oc,ic,ky,kx] W1[ic,d]
  - nearest upsample-by-64 + softmax == softmax of the low-res [64,64] map;
    output row X depends only on low-res index x = X.
  - v enters only through 64-wide block sums: vbar = Wv @ fbar,
      fbar[d,J] = sum_y f[d,J,y]
  - out[c,X,Y] = (sum_J e[J,X] vbar[c,J]) / (64 * sum_J e[J,X]),
      e[J,I] = exp(scale * q_I . k_J), broadcast along Y.

Device kernel (per core = one head), all fp16 on the PE:
  - ONE dma_start per HWDGE ring (the trailing sem-write descriptor blocks
    the engine ring ~1.2us, so extra dma_starts serialize badly): sync ring
    carries f[:, :3072], scalar ring carries f[:, 3072:] ++ w_eff packed.
  - conv as 121 per-tap matmuls: stationary w_eff[:, tap, 16(q8|k8)] fp16,
    moving = strided f slice; 4-way tile_position column packing, all four
    groups accumulating in ONE psum bank (per-partition zero regions).
    No input padding: border taps use restricted oy/ox ranges.
  - PE warm-up: dummy bf16 matmuls during the DMA wait flip the HAM clock
    gate to 2.4 GHz before the real work.
  - fbar reduced on DVE in fp16 while PE does the conv.
  - GELU via the tanh approximation (DVE polynomial + ACT Tanh) so the ACT
    engine only ever needs the exp_and_others table: no 1.3us mid-kernel
    ACT table reload.  The x0.5 of both gelus folds into the exp scale.
  - normalize+broadcast fused in one DVE tensor_scalar with a stride-0 y
    read straight from PSUM; contiguous [64,512] store (host reorders).
"""

import numpy as np

N_CORES = 8
SCALE = 8.0 ** -0.5  # dim_head ** -0.5
C_TANH = 0.7978845608028654  # sqrt(2/pi)
A_TANH = 0.044715

_CACHE = {}
LAST_RESULTS = None  # BassKernelResults of the most recent run (for test harness)

# tap order: 4 full-rectangle interior taps first (one per column group, so
# each group's start=True matmul covers its full [16,64] region)
_HEAD_TAPS = [(5, 3), (5, 4), (5, 5), (5, 6)]
TAPS = _HEAD_TAPS + [
    (ky, kx)
    for ky in range(11)
    for kx in range(11)
    if (ky, kx) not in _HEAD_TAPS
]

N_DUMMY = 12  # HAM warm-up matmuls
CW = 4096  # big_t column offset of the w_eff pack


def _rng(kidx):
    """Valid output range [o0, o1) and first input row for kernel offset."""
    o0 = 1 if kidx < 2 else 0
    o1 = 7 if kidx == 10 else 8
    r0 = 8 * o0 + kidx - 2
    return o0, o1, r0


def _dep(after, before, sync=False):
    from concourse.tile import add_dep_helper

    a = getattr(after, "ins", after)
    b = getattr(before, "ins", before)
    add_dep_helper(a, b, sync=sync, reason="pin order")


def _build_nc():
    from contextlib import ExitStack

    import concourse.bacc as bacc
    import concourse.bass as bass
    import concourse.mybir as mybir
    import concourse.tile as tile

    f32 = mybir.dt.float32
    f16 = mybir.dt.float16
    bf16 = mybir.dt.bfloat16
    X = mybir.AxisListType.X
    AF = mybir.ActivationFunctionType
    ALU = mybir.AluOpType

    nc = bacc.Bacc("TRN2", target_bir_lowering=False)

    s1_d = nc.dram_tensor("s1", [64, 3072], f16, kind="ExternalInput")
    s2_d = nc.dram_tensor("s2", [64, 3072], f16, kind="ExternalInput")
    wE_d = nc.dram_tensor("wE", [128, 16], f16, kind="ExternalInput")
    wb_d = nc.dram_tensor("wb", [8, 2], f32, kind="ExternalInput")
    out_d = nc.dram_tensor("out", [64, 512], f32, kind="ExternalOutput")

    with tile.TileContext(nc) as tc:
        with ExitStack() as ctx:
            sb = ctx.enter_context(tc.tile_pool(name="sb", bufs=1))
            ps = ctx.enter_context(tc.tile_pool(name="ps", bufs=1, space="PSUM"))

            big_t = sb.tile([64, 6144], f16)  # f | w_eff pack | pad
            wE_t = sb.tile([128, 16], f16)
            wb_t = sb.tile([8, 2], f32)
            fbar_t = sb.tile([64, 64], f16)
            S_t = sb.tile([128, 64], f16)
            x_q = sb.tile([8, 64], f16)
            x_k = sb.tile([8, 64], f16)
            t2_q = sb.tile([8, 64], f16)
            t2_k = sb.tile([8, 64], f16)
            h_q = sb.tile([8, 64], f16)
            h_k = sb.tile([8, 64], f16)
            q_t = sb.tile([8, 64], f16)
            k_t = sb.tile([8, 64], f16)
            e_t = sb.tile([64, 64], f16)
            vaug_t = sb.tile([64, 9], f16)
            rs_t = sb.tile([64, 1], f32)
            T_t = sb.tile([64, 8 * 64], f32)
            scr_t = sb.tile([1, 1], f32)
            scr2_t = sb.tile([1, 1], f32)
            dmw_t = sb.tile([64, 128], bf16)
            dmx_t = sb.tile([64, 256], bf16)

            # --- one DMA per ring; small wE/wb behind the sync stream
            nc.sync.dma_start(out=big_t[:, 0:3072], in_=s1_d[:])
            d_s2 = nc.scalar.dma_start(out=big_t[:, 3072:6144], in_=s2_d[:])
            nc.sync.dma_start(out=wE_t, in_=wE_d[:])
            nc.sync.dma_start(out=wb_t, in_=wb_d[:])

            # --- DVE constants + dummy sources (run during DMA wait)
            m_dw = nc.vector.memset(dmw_t, 0.0)
            m_dx = nc.vector.memset(dmx_t, 0.0)
            nc.vector.memset(vaug_t[:, 8:9], 64.0)
            nc.vector.memset(scr_t, 0.0)

            # --- conv accumulator: ONE psum bank; zero the 16-row gaps the
            # column groups leave so the single S-copy reads clean data
            pc = ps.tile([128, 64], f32, tag="A")
            pc4 = pc.rearrange("p (x y) -> p x y", y=8)
            gap_ms = [nc.vector.memset(pc, 0.0)]

            # --- ACT exp-table preload AFTER the scalar-ring DMA trigger
            de = nc.scalar.activation(out=scr2_t, in_=scr_t, func=AF.Exp)
            _dep(de, d_s2)

            # --- HAM warm-up: dummy bf16 matmuls keep PE busy ~3.4us so the
            # clock gate opens to 2.4 GHz before the conv starts
            pd_t = ps.tile([128, 256], f32, tag="E")
            dmy = None
            for i in range(N_DUMMY):
                dmy = nc.tensor.matmul(
                    pd_t, dmw_t, dmx_t, start=True, stop=True,
                    skip_group_check=True,
                )
                if i == 0:
                    _dep(dmy, m_dw)
                    _dep(dmy, m_dx)

            # --- fbar[d, x] = sum_y f[d, x, y] on DVE (fp16, overlaps conv)
            f3 = big_t[:, 0:4096].rearrange("p (x y) -> p x y", y=64)
            with nc.allow_low_precision("fp16 block-sum; 2e-2 rel-err budget"):
                for j in range(4):
                    nc.vector.reduce_sum(
                        out=fbar_t[:, 16 * j : 16 * (j + 1)],
                        in_=f3[:, 16 * j : 16 * j + 16, :],
                        axis=X,
                    )

            # --- conv: 121 taps, 4-way column packing, PSUM accumulate
            totals = [len(range(g, 121, 4)) for g in range(4)]
            seen = [0, 0, 0, 0]
            for t_i, (ky, kx) in enumerate(TAPS):
                g = t_i % 4
                oy0, oy1, ry0 = _rng(ky)
                ox0, ox1, cx0 = _rng(kx)
                n_oy, n_ox = oy1 - oy0, ox1 - ox0
                rhs = f3[
                    :,
                    ry0 : ry0 + 8 * (n_oy - 1) + 1 : 8,
                    cx0 : cx0 + 8 * (n_ox - 1) + 1 : 8,
                ]
                outap = pc4[32 * g : 32 * g + 16, oy0:oy1, ox0:ox1]
                seen[g] += 1
                mm = nc.tensor.matmul(
                    outap,
                    big_t[:, CW + 16 * t_i : CW + 16 * t_i + 16],
                    rhs,
                    start=(seen[g] == 1),
                    stop=(seen[g] == totals[g]),
                    tile_position=(0, 32 * g),
                )
                if t_i == 0:
                    _dep(mm, dmy)
                    for gm in gap_ms:
                        _dep(mm, gm)

            # --- single PSUM -> SBUF copy of all conv partials
            cp_s = nc.vector.tensor_copy(out=S_t, in_=pc)

            # --- vbar while DVE copies S: vaug[J, c] = sum_d fbar[d,J] wvt[d,c]
            psv = ps.tile([64, 8], f32, tag="B")
            nc.tensor.matmul(
                psv, fbar_t, big_t[:, CW + 1936 : CW + 1944],
                start=True, stop=True,
            )
            nc.scalar.copy(out=vaug_t[:, 0:8], in_=psv)

            # --- combine the 4 column groups: q = Eq.T @ S, k = Ek.T @ S
            psq2 = ps.tile([8, 64], f32, tag="F")
            psk2 = ps.tile([8, 64], f32, tag="G")
            nc.tensor.matmul(psq2, wE_t[:, 0:8], S_t, start=True, stop=True)
            nc.tensor.matmul(psk2, wE_t[:, 8:16], S_t, start=True, stop=True)

            # --- 2*gelu(x) = x*(1+tanh(c*(x + a*x^3))); the 0.5 of both
            # branches folds into the exp scale (exp(S/4 * dots'))
            def gelu2(psx, bias, x_t, t2_t, h_t, out_t):
                nc.vector.tensor_scalar_add(x_t, psx, bias)
                nc.vector.tensor_mul(t2_t, x_t, x_t)
                nc.vector.tensor_scalar(
                    out=t2_t, in0=t2_t, scalar1=A_TANH, scalar2=1.0,
                    op0=ALU.mult, op1=ALU.add,
                )
                nc.vector.tensor_mul(t2_t, t2_t, x_t)
                nc.scalar.activation(out=h_t, in_=t2_t, func=AF.Tanh, scale=C_TANH)
                nc.vector.scalar_tensor_tensor(
                    out=out_t, in0=h_t, scalar=1.0, in1=x_t,
                    op0=ALU.add, op1=ALU.mult,
                )

            gelu2(psq2, wb_t[:, 0:1], x_q, t2_q, h_q, q_t)
            gelu2(psk2, wb_t[:, 1:2], x_k, t2_k, h_k, k_t)

            # --- dots_T[J, I] = sum_c k[c,J] q[c,I]; e = exp(S/4 * dots_T)
            psd = ps.tile([64, 64], f32, tag="H")
            nc.tensor.matmul(psd, k_t, q_t, start=True, stop=True)
            nc.scalar.activation(out=e_t, in_=psd, func=AF.Exp, scale=SCALE / 4)

            # --- out_u[I, 0:8] = sum_J e[J,I] vaug[J,c]; col 8 = 64*sum_J e
            pso = ps.tile([64, 9], f32, tag="C")
            nc.tensor.matmul(pso, e_t, vaug_t, start=True, stop=True)
            nc.vector.reciprocal(out=rs_t, in_=pso[:, 8:9])

            # --- fused normalize + broadcast along y (stride-0 read on y,
            # straight from PSUM), then contiguous store (host reorders)
            T3 = T_t.rearrange("p (c y) -> p c y", y=64)
            ola = pso[:, 0:8]
            ol_b = bass.AP(
                tensor=ola.tensor, offset=ola.offset,
                ap=[list(ola.ap[0]), list(ola.ap[1]), [0, 64]],
            )
            nc.vector.tensor_scalar_mul(T3, ol_b, rs_t)
            nc.sync.dma_start(out=out_d[:], in_=T_t[:])

    nc.finalize()
    return nc


def _get_nc():
    if "nc" not in _CACHE:
        _CACHE["nc"] = _build_nc()
    return _CACHE["nc"]


def kernel(**inputs):
    global LAST_RESULTS
    from concourse.bass_utils import run_bass_kernel_spmd

    f = np.ascontiguousarray(inputs["f"], np.float32)
    w_qkv = np.ascontiguousarray(inputs["w_qkv"], np.float32)[:, :, 0, 0]
    wq = np.ascontiguousarray(inputs["wq"], np.float32)
    wk = np.ascontiguousarray(inputs["wk"], np.float32)
    bq = np.ascontiguousarray(inputs["bq"], np.float32)
    bk = np.ascontiguousarray(inputs["bk"], np.float32)

    W1q, W1k, Wv = w_qkv[0:64], w_qkv[64:128], w_qkv[128:192]
    # w_eff[ky, kx, oc, d] = sum_ic w[oc, ic, ky, kx] * W1[ic, d]
    weq = np.einsum("oikl,id->klod", wq, W1q).astype(np.float16)
    wek = np.einsum("oikl,id->klod", wk, W1k).astype(np.float16)

    f16 = f[0].reshape(64, 4096).astype(np.float16)
    s1 = np.ascontiguousarray(f16[:, 0:3072])

    in_maps = []
    for i in range(N_CORES):
        sl = slice(8 * i, 8 * i + 8)
        s2 = np.zeros((64, 3072), np.float16)
        s2[:, 0:1024] = f16[:, 3072:4096]
        for t_i, (ky, kx) in enumerate(TAPS):
            s2[:, 1024 + 16 * t_i : 1024 + 16 * t_i + 8] = weq[ky, kx, sl].T
            s2[:, 1024 + 16 * t_i + 8 : 1024 + 16 * t_i + 16] = wek[
                ky, kx, sl
            ].T
        s2[:, 1024 + 1936 : 1024 + 1944] = Wv[sl].T.astype(np.float16)
        wE = np.zeros((128, 16), np.float16)
        for g in range(4):
            for c in range(8):
                wE[32 * g + c, c] = 1.0
                wE[32 * g + 8 + c, 8 + c] = 1.0
        wb = np.stack([bq[sl], bk[sl]], axis=1).astype(np.float32)
        in_maps.append({"s1": s1, "s2": s2, "wE": wE, "wb": wb})

    nc = _get_nc()
    res = run_bass_kernel_spmd(nc, in_maps, core_ids=list(range(N_CORES)))
    LAST_RESULTS = res
    outs = []
    for r in res.results:
        t = r["out"].reshape(64, 8, 64).transpose(1, 0, 2)  # [c, x, y]
        outs.append(t.reshape(8, 4096))
    out = np.concatenate(outs, axis=0)  # [64, 4096]
    return out.reshape(1, 64, 64, 64)


# revision 9
# speedup vs baseline: 1.4206x; 1.1664x over previous
"""Trainium2 Bass kernel for nn_Attention_32049045963483 (sparse_attention).

Math collapse (verified vs reference: ~3e-6 rel err fp32, ~5e-4 fp16):
  - qkv 1x1 conv folds into the 11x11/stride-8 down-convs (host-side fold):
      w_eff[d, ky, kx, oc] = sum_ic w[oc,ic,ky,kx] W1[ic,d]
  - nearest upsample-by-64 + softmax == softmax of the low-res [64,64] map;
    output row X depends only on low-res index x = X.
  - v enters only through 64-wide block sums: vbar = Wv @ fbar,
      fbar[d,J] = sum_y f[d,J,y]
  - out[c,X,Y] = (sum_J e[J,X] vbar[c,J]) / (64 * sum_J e[J,X]),
      e[J,I] = exp(scale * q_I . k_J), broadcast along Y.

Device kernel (per core = one head), all fp16 on the PE:
  - one dma_start per HWDGE ring (extra dma_starts stall the engine ring
    ~1.2us on the trailing sem-write descriptor), 4KB/8KB-aligned rows
    (6KB rows measured ~2x slower per engine).  SDMA engines round-robin
    rings at packet granularity, so per-engine bytes are the bandwidth cap.
  - conv as 121 per-tap matmuls: stationary w_eff[:, tap, 16(q8|k8)] fp16,
    moving = strided f slice; 4-way tile_position column packing, all four
    groups accumulating in ONE psum bank (per-partition zero regions).
    No input padding: border taps use restricted oy/ox ranges.
  - PE warm-up: dummy bf16 matmuls during the DMA wait flip the HAM clock
    gate to 2.4 GHz before the real work.
  - fbar reduced on DVE in fp16 while PE does the conv (hidden).
  - q|k fused in one [8,128] psum tile (one accumulation group, 3 matmuls:
    Eq-combine, Ek-combine, bias via a 2-row indicator matmul), so the
    tanh-approx GELU chain runs once at [8,128] (DVE overhead dominates at
    this size).  ACT only ever needs the exp_and_others table (tanh+exp):
    no mid-kernel ACT table reload.  The x0.5 of both gelus folds into the
    exp scale.
  - contiguous [64,512] store; host reorders [x, c, y] -> [c, x, y].
"""

import numpy as np

N_CORES = 8
SCALE = 8.0 ** -0.5  # dim_head ** -0.5
C_TANH = 0.7978845608028654  # sqrt(2/pi)
A_TANH = 0.044715

_CACHE = {}
LAST_RESULTS = None  # BassKernelResults of the most recent run (for test harness)

# tap order: 4 full-rectangle interior taps first (one per column group, so
# each group's start=True matmul covers its full [16,64] region)
_HEAD_TAPS = [(5, 3), (5, 4), (5, 5), (5, 6)]
TAPS = _HEAD_TAPS + [
    (ky, kx)
    for ky in range(11)
    for kx in range(11)
    if (ky, kx) not in _HEAD_TAPS
]

N_DUMMY = 12  # HAM warm-up matmuls
CW = 4096  # big_t column offset of the w_eff pack


def _rng(kidx):
    """Valid output range [o0, o1) and first input row for kernel offset."""
    o0 = 1 if kidx < 2 else 0
    o1 = 7 if kidx == 10 else 8
    r0 = 8 * o0 + kidx - 2
    return o0, o1, r0


def _dep(after, before, sync=False):
    from concourse.tile import add_dep_helper

    a = getattr(after, "ins", after)
    b = getattr(before, "ins", before)
    add_dep_helper(a, b, sync=sync, reason="pin order")


def _build_nc():
    from contextlib import ExitStack

    import concourse.bacc as bacc
    import concourse.bass as bass
    import concourse.mybir as mybir
    import concourse.tile as tile

    f32 = mybir.dt.float32
    f16 = mybir.dt.float16
    bf16 = mybir.dt.bfloat16
    X = mybir.AxisListType.X
    AF = mybir.ActivationFunctionType
    ALU = mybir.AluOpType

    nc = bacc.Bacc("TRN2", target_bir_lowering=False)

    s1_d = nc.dram_tensor("s1", [64, 2048], f16, kind="ExternalInput")
    s2_d = nc.dram_tensor("s2", [64, 4096], f16, kind="ExternalInput")
    wE_d = nc.dram_tensor("wE", [128, 16], f16, kind="ExternalInput")
    wc_d = nc.dram_tensor("wc", [2, 136], f16, kind="ExternalInput")
    out_d = nc.dram_tensor("out", [64, 512], f32, kind="ExternalOutput")

    with tile.TileContext(nc) as tc:
        with ExitStack() as ctx:
            sb = ctx.enter_context(tc.tile_pool(name="sb", bufs=1))
            ps = ctx.enter_context(tc.tile_pool(name="ps", bufs=1, space="PSUM"))

            big_t = sb.tile([64, 6144], f16)  # f | w_eff pack | pad
            wE_t = sb.tile([128, 16], f16)
            wc_t = sb.tile([2, 136], f16)
            fbar_t = sb.tile([64, 64], f16)
            S_t = sb.tile([128, 64], f16)
            xk_t = sb.tile([8, 128], f16)
            t2_t = sb.tile([8, 128], f16)
            h2_t = sb.tile([8, 128], f16)
            qk_t = sb.tile([8, 128], f16)
            e_t = sb.tile([64, 64], f16)
            vaug_t = sb.tile([64, 9], f16)
            rs_t = sb.tile([64, 1], f32)
            olT_t = sb.tile([64, 8], f32)
            T_t = sb.tile([64, 8 * 64], f32)
            scr_t = sb.tile([1, 1], f32)
            scr2_t = sb.tile([1, 1], f32)
            dmw_t = sb.tile([64, 128], bf16)
            dmx_t = sb.tile([64, 256], bf16)

            # --- one DMA per ring; small wE/wc behind the sync stream
            nc.sync.dma_start(out=big_t[:, 0:2048], in_=s1_d[:])
            d_s2 = nc.scalar.dma_start(out=big_t[:, 2048:6144], in_=s2_d[:])
            nc.sync.dma_start(out=wE_t, in_=wE_d[:])
            nc.sync.dma_start(out=wc_t, in_=wc_d[:])

            # --- DVE constants + dummy sources (run during DMA wait)
            m_dw = nc.vector.memset(dmw_t, 0.0)
            m_dx = nc.vector.memset(dmx_t, 0.0)
            nc.vector.memset(vaug_t[:, 8:9], 64.0)
            nc.vector.memset(scr_t, 0.0)

            # --- conv accumulator: ONE psum bank; zero it so the 16-row gaps
            # the column groups leave read back clean in the single S-copy
            pc = ps.tile([128, 64], f32, tag="A")
            pc4 = pc.rearrange("p (x y) -> p x y", y=8)
            gap_ms = [nc.vector.memset(pc, 0.0)]

            # --- ACT exp-table preload AFTER the scalar-ring DMA trigger
            de = nc.scalar.activation(out=scr2_t, in_=scr_t, func=AF.Exp)
            _dep(de, d_s2)

            # --- HAM warm-up: dummy bf16 matmuls keep PE busy ~3.4us so the
            # clock gate opens to 2.4 GHz before the conv starts
            pd_t = ps.tile([128, 256], f32, tag="E")
            dmy = None
            for i in range(N_DUMMY):
                dmy = nc.tensor.matmul(
                    pd_t, dmw_t, dmx_t, start=True, stop=True,
                    skip_group_check=True,
                )
                if i == 0:
                    _dep(dmy, m_dw)
                    _dep(dmy, m_dx)

            # --- fbar[d, x] = sum_y f[d, x, y] on DVE (fp16, overlaps conv)
            f3 = big_t[:, 0:4096].rearrange("p (x y) -> p x y", y=64)
            with nc.allow_low_precision("fp16 block-sum; 2e-2 rel-err budget"):
                for j in range(4):
                    nc.vector.reduce_sum(
                        out=fbar_t[:, 16 * j : 16 * (j + 1)],
                        in_=f3[:, 16 * j : 16 * j + 16, :],
                        axis=X,
                    )

            # --- conv: 121 taps, 4-way column packing, PSUM accumulate
            totals = [len(range(g, 121, 4)) for g in range(4)]
            seen = [0, 0, 0, 0]
            for t_i, (ky, kx) in enumerate(TAPS):
                g = t_i % 4
                oy0, oy1, ry0 = _rng(ky)
                ox0, ox1, cx0 = _rng(kx)
                n_oy, n_ox = oy1 - oy0, ox1 - ox0
                rhs = f3[
                    :,
                    ry0 : ry0 + 8 * (n_oy - 1) + 1 : 8,
                    cx0 : cx0 + 8 * (n_ox - 1) + 1 : 8,
                ]
                outap = pc4[32 * g : 32 * g + 16, oy0:oy1, ox0:ox1]
                seen[g] += 1
                mm = nc.tensor.matmul(
                    outap,
                    big_t[:, CW + 16 * t_i : CW + 16 * t_i + 16],
                    rhs,
                    start=(seen[g] == 1),
                    stop=(seen[g] == totals[g]),
                    tile_position=(0, 32 * g),
                )
                if t_i == 0:
                    _dep(mm, dmy)
                    for gm in gap_ms:
                        _dep(mm, gm)

            # --- single PSUM -> SBUF copy of all conv partials
            nc.vector.tensor_copy(out=S_t, in_=pc)

            # --- vbar while DVE copies S: vaug[J, c] = sum_d fbar[d,J] wvt[d,c]
            psv = ps.tile([64, 8], f32, tag="B")
            nc.tensor.matmul(
                psv, fbar_t, big_t[:, CW + 1936 : CW + 1944],
                start=True, stop=True,
            )
            nc.scalar.copy(out=vaug_t[:, 0:8], in_=psv)

            # --- combine column groups + bias, q|k fused in one [8,128] bank:
            # cols 0:64 = q + bq, cols 64:128 = k + bk (bias via 2-row matmul)
            psqk = ps.tile([8, 128], f32, tag="F")
            nc.tensor.matmul(
                psqk[:, 0:64], wE_t[:, 0:8], S_t, start=True, stop=False
            )
            nc.tensor.matmul(
                psqk[:, 64:128], wE_t[:, 8:16], S_t, start=False, stop=False
            )
            nc.tensor.matmul(
                psqk, wc_t[:, 0:8], wc_t[:, 8:136], start=False, stop=True
            )

            # --- 2*gelu(x) = x*(1+tanh(c*(x + a*x^3))) on [8,128]; the 0.5
            # of both branches folds into the exp scale (exp(S/4 * dots'))
            nc.vector.tensor_copy(out=xk_t, in_=psqk)
            nc.vector.tensor_mul(t2_t, xk_t, xk_t)
            nc.vector.tensor_scalar(
                out=t2_t, in0=t2_t, scalar1=A_TANH, scalar2=1.0,
                op0=ALU.mult, op1=ALU.add,
            )
            nc.vector.tensor_mul(t2_t, t2_t, xk_t)
            nc.scalar.activation(out=h2_t, in_=t2_t, func=AF.Tanh, scale=C_TANH)
            nc.vector.scalar_tensor_tensor(
                out=qk_t, in0=h2_t, scalar=1.0, in1=xk_t,
                op0=ALU.add, op1=ALU.mult,
            )

            # --- dots_T[J, I] = sum_c k[c,J] q[c,I]; e = exp(S/4 * dots_T)
            psd = ps.tile([64, 64], f32, tag="H")
            nc.tensor.matmul(
                psd, qk_t[:, 64:128], qk_t[:, 0:64], start=True, stop=True
            )
            nc.scalar.activation(out=e_t, in_=psd, func=AF.Exp, scale=SCALE / 4)

            # --- out_u[I, 0:8] = sum_J e[J,I] vaug[J,c]; col 8 = 64*sum_J e
            pso = ps.tile([64, 9], f32, tag="C")
            nc.tensor.matmul(pso, e_t, vaug_t, start=True, stop=True)
            nc.vector.reciprocal(out=rs_t, in_=pso[:, 8:9])
            nc.vector.tensor_scalar_mul(olT_t, pso[:, 0:8], rs_t)

            # --- broadcast along y: single DVE copy with stride-0 read on y
            T3 = T_t.rearrange("p (c y) -> p c y", y=64)
            ola = olT_t[:]
            ol_b = bass.AP(
                tensor=ola.tensor, offset=ola.offset,
                ap=[list(ola.ap[0]), list(ola.ap[1]), [0, 64]],
            )
            nc.vector.tensor_copy(out=T3, in_=ol_b)

            # --- contiguous store; host reorders [x, c, y] -> [c, x, y]
            nc.sync.dma_start(out=out_d[:], in_=T_t[:])

    nc.finalize()
    return nc


def _get_nc():
    if "nc" not in _CACHE:
        _CACHE["nc"] = _build_nc()
    return _CACHE["nc"]


def kernel(**inputs):
    global LAST_RESULTS
    from concourse.bass_utils import run_bass_kernel_spmd

    f = np.ascontiguousarray(inputs["f"], np.float32)
    w_qkv = np.ascontiguousarray(inputs["w_qkv"], np.float32)[:, :, 0, 0]
    wq = np.ascontiguousarray(inputs["wq"], np.float32)
    wk = np.ascontiguousarray(inputs["wk"], np.float32)
    bq = np.ascontiguousarray(inputs["bq"], np.float32)
    bk = np.ascontiguousarray(inputs["bk"], np.float32)

    W1q, W1k, Wv = w_qkv[0:64], w_qkv[64:128], w_qkv[128:192]
    # w_eff[ky, kx, oc, d] = sum_ic w[oc, ic, ky, kx] * W1[ic, d]
    weq = np.einsum("oikl,id->klod", wq, W1q).astype(np.float16)
    wek = np.einsum("oikl,id->klod", wk, W1k).astype(np.float16)

    f16 = f[0].reshape(64, 4096).astype(np.float16)
    s1 = np.ascontiguousarray(f16[:, 0:2048])

    in_maps = []
    for i in range(N_CORES):
        sl = slice(8 * i, 8 * i + 8)
        s2 = np.zeros((64, 4096), np.float16)
        s2[:, 0:2048] = f16[:, 2048:4096]
        for t_i, (ky, kx) in enumerate(TAPS):
            s2[:, 2048 + 16 * t_i : 2048 + 16 * t_i + 8] = weq[ky, kx, sl].T
            s2[:, 2048 + 16 * t_i + 8 : 2048 + 16 * t_i + 16] = wek[
                ky, kx, sl
            ].T
        s2[:, 2048 + 1936 : 2048 + 1944] = Wv[sl].T.astype(np.float16)
        wE = np.zeros((128, 16), np.float16)
        for g in range(4):
            for c in range(8):
                wE[32 * g + c, c] = 1.0
                wE[32 * g + 8 + c, 8 + c] = 1.0
        wc = np.zeros((2, 136), np.float16)
        wc[0, 0:8] = bq[sl]
        wc[1, 0:8] = bk[sl]
        wc[0, 8 : 8 + 64] = 1.0
        wc[1, 8 + 64 : 8 + 128] = 1.0
        in_maps.append({"s1": s1, "s2": s2, "wE": wE, "wc": wc})

    nc = _get_nc()
    res = run_bass_kernel_spmd(nc, in_maps, core_ids=list(range(N_CORES)))
    LAST_RESULTS = res
    outs = []
    for r in res.results:
        t = r["out"].reshape(64, 8, 64).transpose(1, 0, 2)  # [c, x, y]
        outs.append(t.reshape(8, 4096))
    out = np.concatenate(outs, axis=0)  # [64, 4096]
    return out.reshape(1, 64, 64, 64)


# revision 10
# speedup vs baseline: 1.4578x; 1.0262x over previous
"""Trainium2 Bass kernel for nn_Attention_32049045963483 (sparse_attention).

Math collapse (verified vs reference: ~3e-6 rel err fp32, ~5e-4 fp16):
  - qkv 1x1 conv folds into the 11x11/stride-8 down-convs (host-side fold):
      w_eff[d, ky, kx, oc] = sum_ic w[oc,ic,ky,kx] W1[ic,d]
  - nearest upsample-by-64 + softmax == softmax of the low-res [64,64] map;
    output row X depends only on low-res index x = X.
  - v enters only through 64-wide block sums: vbar = Wv @ fbar,
      fbar[d,J] = sum_y f[d,J,y]
  - out[c,X,Y] = (sum_J e[J,X] vbar[c,J]) / (64 * sum_J e[J,X]),
      e[J,I] = exp(scale * q_I . k_J), broadcast along Y.

Device kernel (per core = one head), all fp16 on the PE:
  - one dma_start per HWDGE ring (extra dma_starts stall the engine ring
    ~1.2us on the trailing sem-write descriptor), 4KB/8KB-aligned rows
    (6KB rows measured ~2x slower per engine).  SDMA engines round-robin
    rings at packet granularity, so per-engine bytes are the bandwidth cap.
  - conv as 121 per-tap matmuls: stationary w_eff[:, tap, 16(q8|k8)] fp16,
    moving = strided f slice; 4-way tile_position column packing, all four
    groups accumulating in ONE psum bank (per-partition zero regions).
    No input padding: border taps use restricted oy/ox ranges.
  - PE warm-up: dummy bf16 matmuls during the DMA wait flip the HAM clock
    gate to 2.4 GHz before the real work.
  - fbar reduced on DVE in fp16 while PE does the conv (hidden).
  - q|k fused in one [8,128] psum tile (one accumulation group, 3 matmuls:
    Eq-combine, Ek-combine, bias via a 2-row indicator matmul), so the
    tanh-approx GELU chain runs once at [8,128] (DVE overhead dominates at
    this size).  ACT only ever needs the exp_and_others table (tanh+exp):
    no mid-kernel ACT table reload.  The x0.5 of both gelus folds into the
    exp scale.
  - contiguous [64,512] store; host reorders [x, c, y] -> [c, x, y].
"""

import numpy as np

N_CORES = 8
SCALE = 8.0 ** -0.5  # dim_head ** -0.5
C_TANH = 0.7978845608028654  # sqrt(2/pi)
A_TANH = 0.044715

_CACHE = {}
LAST_RESULTS = None  # BassKernelResults of the most recent run (for test harness)

# tap order: 4 full-rectangle interior taps first (one per column group, so
# each group's start=True matmul covers its full [16,64] region)
_HEAD_TAPS = [(5, 3), (5, 4), (5, 5), (5, 6)]
TAPS = _HEAD_TAPS + [
    (ky, kx)
    for ky in range(11)
    for kx in range(11)
    if (ky, kx) not in _HEAD_TAPS
]

N_DUMMY = 12  # HAM warm-up matmuls
CW = 4096  # big_t column offset of the w_eff pack


def _rng(kidx):
    """Valid output range [o0, o1) and first input row for kernel offset."""
    o0 = 1 if kidx < 2 else 0
    o1 = 7 if kidx == 10 else 8
    r0 = 8 * o0 + kidx - 2
    return o0, o1, r0


def _dep(after, before, sync=False):
    from concourse.tile import add_dep_helper

    a = getattr(after, "ins", after)
    b = getattr(before, "ins", before)
    add_dep_helper(a, b, sync=sync, reason="pin order")


def _build_nc():
    from contextlib import ExitStack

    import concourse.bacc as bacc
    import concourse.bass as bass
    import concourse.mybir as mybir
    import concourse.tile as tile

    f32 = mybir.dt.float32
    f16 = mybir.dt.float16
    bf16 = mybir.dt.bfloat16
    X = mybir.AxisListType.X
    AF = mybir.ActivationFunctionType
    ALU = mybir.AluOpType

    nc = bacc.Bacc("TRN2", target_bir_lowering=False)

    s1_d = nc.dram_tensor("s1", [64, 2048], f16, kind="ExternalInput")
    s2_d = nc.dram_tensor("s2", [64, 4096], f16, kind="ExternalInput")
    wE_d = nc.dram_tensor("wE", [128, 16], f16, kind="ExternalInput")
    wc_d = nc.dram_tensor("wc", [2, 136], f16, kind="ExternalInput")
    out_d = nc.dram_tensor("out", [64, 512], f32, kind="ExternalOutput")

    with tile.TileContext(nc) as tc:
        with ExitStack() as ctx:
            sb = ctx.enter_context(tc.tile_pool(name="sb", bufs=1))
            ps = ctx.enter_context(tc.tile_pool(name="ps", bufs=1, space="PSUM"))

            big_t = sb.tile([64, 6144], f16)  # f | w_eff pack | pad
            wE_t = sb.tile([128, 16], f16)
            wc_t = sb.tile([2, 136], f16)
            fbar_t = sb.tile([64, 64], f16)
            S_t = sb.tile([128, 64], f16)
            h2_t = sb.tile([8, 128], f16)
            qk_t = sb.tile([8, 128], f16)
            e_t = sb.tile([64, 64], f16)
            vaug_t = sb.tile([64, 9], f16)
            rs_t = sb.tile([64, 1], f32)
            olT_t = sb.tile([64, 8], f32)
            T_t = sb.tile([64, 8 * 64], f32)
            scr_t = sb.tile([1, 1], f32)
            scr2_t = sb.tile([1, 1], f32)
            dmw_t = sb.tile([64, 128], bf16)
            dmx_t = sb.tile([64, 256], bf16)

            # --- one DMA per ring; small wE/wc behind the sync stream
            nc.sync.dma_start(out=big_t[:, 0:2048], in_=s1_d[:])
            d_s2 = nc.scalar.dma_start(
                out=big_t[:, 2048:6144].rearrange("p (c k) -> p c k", k=2048),
                in_=s2_d[:].rearrange("p (c k) -> p c k", k=2048),
            )
            nc.sync.dma_start(out=wE_t, in_=wE_d[:])
            nc.sync.dma_start(out=wc_t, in_=wc_d[:])

            # --- DVE constants + dummy sources (run during DMA wait)
            m_dw = nc.vector.memset(dmw_t, 0.0)
            m_dx = nc.vector.memset(dmx_t, 0.0)
            nc.vector.memset(vaug_t[:, 8:9], 64.0)
            nc.vector.memset(scr_t, 0.0)

            # --- conv accumulator: ONE psum bank; zero it so the 16-row gaps
            # the column groups leave read back clean in the single S-copy
            pc = ps.tile([128, 64], f32, tag="A")
            pc4 = pc.rearrange("p (x y) -> p x y", y=8)
            gap_ms = [nc.vector.memset(pc, 0.0)]

            # --- ACT exp-table preload AFTER the scalar-ring DMA trigger
            de = nc.scalar.activation(out=scr2_t, in_=scr_t, func=AF.Exp)
            _dep(de, d_s2)

            # --- HAM warm-up: dummy bf16 matmuls keep PE busy ~3.4us so the
            # clock gate opens to 2.4 GHz before the conv starts
            pd_t = ps.tile([128, 256], f32, tag="E")
            dmy = None
            for i in range(N_DUMMY):
                dmy = nc.tensor.matmul(
                    pd_t, dmw_t, dmx_t, start=True, stop=True,
                    skip_group_check=True,
                )
                if i == 0:
                    _dep(dmy, m_dw)
                    _dep(dmy, m_dx)

            # --- fbar[d, x] = sum_y f[d, x, y] on DVE (fp16, overlaps conv)
            f3 = big_t[:, 0:4096].rearrange("p (x y) -> p x y", y=64)
            with nc.allow_low_precision("fp16 block-sum; 2e-2 rel-err budget"):
                for j in range(4):
                    nc.vector.reduce_sum(
                        out=fbar_t[:, 16 * j : 16 * (j + 1)],
                        in_=f3[:, 16 * j : 16 * j + 16, :],
                        axis=X,
                    )

            # --- conv: 121 taps, 4-way column packing, PSUM accumulate
            totals = [len(range(g, 121, 4)) for g in range(4)]
            seen = [0, 0, 0, 0]
            for t_i, (ky, kx) in enumerate(TAPS):
                g = t_i % 4
                oy0, oy1, ry0 = _rng(ky)
                ox0, ox1, cx0 = _rng(kx)
                n_oy, n_ox = oy1 - oy0, ox1 - ox0
                rhs = f3[
                    :,
                    ry0 : ry0 + 8 * (n_oy - 1) + 1 : 8,
                    cx0 : cx0 + 8 * (n_ox - 1) + 1 : 8,
                ]
                outap = pc4[32 * g : 32 * g + 16, oy0:oy1, ox0:ox1]
                seen[g] += 1
                mm = nc.tensor.matmul(
                    outap,
                    big_t[:, CW + 16 * t_i : CW + 16 * t_i + 16],
                    rhs,
                    start=(seen[g] == 1),
                    stop=(seen[g] == totals[g]),
                    tile_position=(0, 32 * g),
                )
                if t_i == 0:
                    _dep(mm, dmy)
                    for gm in gap_ms:
                        _dep(mm, gm)

            # --- single PSUM -> SBUF copy of all conv partials
            nc.vector.tensor_copy(out=S_t, in_=pc)

            # --- vbar while DVE copies S: vaug[J, c] = sum_d fbar[d,J] wvt[d,c]
            psv = ps.tile([64, 8], f32, tag="B")
            nc.tensor.matmul(
                psv, fbar_t, big_t[:, CW + 1936 : CW + 1944],
                start=True, stop=True,
            )
            nc.scalar.copy(out=vaug_t[:, 0:8], in_=psv)

            # --- combine column groups + bias, q|k fused in one [8,128] bank:
            # cols 0:64 = q + bq, cols 64:128 = k + bk (bias via 2-row matmul)
            psqk = ps.tile([8, 128], f32, tag="F")
            nc.tensor.matmul(
                psqk[:, 0:64], wE_t[:, 0:8], S_t, start=True, stop=False
            )
            nc.tensor.matmul(
                psqk[:, 64:128], wE_t[:, 8:16], S_t, start=False, stop=False
            )
            nc.tensor.matmul(
                psqk, wc_t[:, 0:8], wc_t[:, 8:136], start=False, stop=True
            )

            # --- 2*gelu(x) ~= x*(1+tanh(0.851x)) on [8,128] (sigmoid form,
            # l2 5e-3 vs 2e-2 budget); the 0.5 folds into the exp scale
            nc.scalar.activation(out=h2_t, in_=psqk, func=AF.Tanh, scale=0.851)
            nc.vector.scalar_tensor_tensor(
                out=qk_t, in0=h2_t, scalar=1.0, in1=psqk,
                op0=ALU.add, op1=ALU.mult,
            )

            # --- dots_T[J, I] = sum_c k[c,J] q[c,I]; e = exp(S/4 * dots_T)
            psd = ps.tile([64, 64], f32, tag="H")
            nc.tensor.matmul(
                psd, qk_t[:, 64:128], qk_t[:, 0:64], start=True, stop=True
            )
            nc.scalar.activation(out=e_t, in_=psd, func=AF.Exp, scale=SCALE / 4)

            # --- out_u[I, 0:8] = sum_J e[J,I] vaug[J,c]; col 8 = 64*sum_J e
            pso = ps.tile([64, 9], f32, tag="C")
            nc.tensor.matmul(pso, e_t, vaug_t, start=True, stop=True)
            nc.vector.reciprocal(out=rs_t, in_=pso[:, 8:9])
            nc.vector.tensor_scalar_mul(olT_t, pso[:, 0:8], rs_t)

            # --- broadcast along y (stride-0 read), split in halves so each
            # ring stores its half as soon as it is ready
            T3 = T_t.rearrange("p (c y) -> p c y", y=64)
            for h, eng in ((0, nc.sync), (1, nc.scalar)):
                ola = olT_t[:, 4 * h : 4 * h + 4]
                ol_b = bass.AP(
                    tensor=ola.tensor, offset=ola.offset,
                    ap=[list(ola.ap[0]), list(ola.ap[1]), [0, 64]],
                )
                nc.vector.tensor_copy(out=T3[:, 4 * h : 4 * h + 4, :], in_=ol_b)
                eng.dma_start(
                    out=out_d[:, 256 * h : 256 * h + 256],
                    in_=T_t[:, 256 * h : 256 * h + 256],
                )

    nc.finalize()
    return nc


def _get_nc():
    if "nc" not in _CACHE:
        _CACHE["nc"] = _build_nc()
    return _CACHE["nc"]


def kernel(**inputs):
    global LAST_RESULTS
    from concourse.bass_utils import run_bass_kernel_spmd

    f = np.ascontiguousarray(inputs["f"], np.float32)
    w_qkv = np.ascontiguousarray(inputs["w_qkv"], np.float32)[:, :, 0, 0]
    wq = np.ascontiguousarray(inputs["wq"], np.float32)
    wk = np.ascontiguousarray(inputs["wk"], np.float32)
    bq = np.ascontiguousarray(inputs["bq"], np.float32)
    bk = np.ascontiguousarray(inputs["bk"], np.float32)

    W1q, W1k, Wv = w_qkv[0:64], w_qkv[64:128], w_qkv[128:192]
    # w_eff[ky, kx, oc, d] = sum_ic w[oc, ic, ky, kx] * W1[ic, d]
    weq = np.einsum("oikl,id->klod", wq, W1q).astype(np.float16)
    wek = np.einsum("oikl,id->klod", wk, W1k).astype(np.float16)

    f16 = f[0].reshape(64, 4096).astype(np.float16)
    s1 = np.ascontiguousarray(f16[:, 0:2048])

    in_maps = []
    for i in range(N_CORES):
        sl = slice(8 * i, 8 * i + 8)
        s2 = np.zeros((64, 4096), np.float16)
        s2[:, 0:2048] = f16[:, 2048:4096]
        for t_i, (ky, kx) in enumerate(TAPS):
            s2[:, 2048 + 16 * t_i : 2048 + 16 * t_i + 8] = weq[ky, kx, sl].T
            s2[:, 2048 + 16 * t_i + 8 : 2048 + 16 * t_i + 16] = wek[
                ky, kx, sl
            ].T
        s2[:, 2048 + 1936 : 2048 + 1944] = Wv[sl].T.astype(np.float16)
        wE = np.zeros((128, 16), np.float16)
        for g in range(4):
            for c in range(8):
                wE[32 * g + c, c] = 1.0
                wE[32 * g + 8 + c, 8 + c] = 1.0
        wc = np.zeros((2, 136), np.float16)
        wc[0, 0:8] = bq[sl]
        wc[1, 0:8] = bk[sl]
        wc[0, 8 : 8 + 64] = 1.0
        wc[1, 8 + 64 : 8 + 128] = 1.0
        in_maps.append({"s1": s1, "s2": s2, "wE": wE, "wc": wc})

    nc = _get_nc()
    res = run_bass_kernel_spmd(nc, in_maps, core_ids=list(range(N_CORES)))
    LAST_RESULTS = res
    outs = []
    for r in res.results:
        t = r["out"].reshape(64, 8, 64).transpose(1, 0, 2)  # [c, x, y]
        outs.append(t.reshape(8, 4096))
    out = np.concatenate(outs, axis=0)  # [64, 4096]
    return out.reshape(1, 64, 64, 64)
